# revision 12
# baseline (speedup 1.0000x reference)
"""Causal segment-masked depthwise conv (K=5) + pointwise conv, 8-core SPMD.

Strategy:
  Host: pack each batch row's segments into a global stream with 4 zeros
  before each segment (plain causal conv on the stream == per-segment
  left-zero-padded conv), split the stream evenly across 8 cores with a
  4-element halo, and pre-transpose each core's slab to [C, stream].
  Device: depthwise conv for channel chunks 0-1 as diag-stationary fp32r
  matmuls in PSUM (+ ACT bias copy), chunks 2-3 on DVE via fused
  scalar_tensor_tensor over 1024-wide superblocks; pointwise matmul with
  w_pw^T chunks stationary and dwT moving -> transposed [d, l] PSUM
  output, ACT adds b_pw as per-partition bias, batched store. Host
  transposes back during gather and applies a sparse general-case
  correction for exotic segment overlap patterns (empty for contiguous
  partitions).
"""

import os
import sys

sys.path.insert(0, "/opt/trn_rl_repo")

import numpy as np

B, L, C, K, S = 8, 4096, 512, 5, 8
NCORES = 8
CCH = C // 128          # 4 channel chunks
XT_W = 4360             # per-core stream buffer: 4 halo + 4348 capacity + pad
OUT_ROWS = 4352         # conv outputs for stream cols [4, 4356)
NBLK = 9                # 8 blocks of 512 + 1 of 256
BLKS = [512] * 8 + [256]
SBS = [1024] * 4 + [256]   # DVE superblocks
PE_CH = 2               # channel chunks 0..PE_CH-1 on PE, rest on DVE

_cached = {}


def _build_nc():
    import concourse.mybir as mybir
    from concourse import bacc
    from concourse.tile import TileContext
    from concourse.masks import make_identity

    f32 = mybir.dt.float32
    f32r = mybir.dt.float32r
    Alu = mybir.AluOpType

    nc = bacc.Bacc(num_swdge_queues=2)
    xt_d = nc.declare_dram_parameter("xt", [C, XT_W], f32, isOutput=False)
    wdiag_d = nc.declare_dram_parameter("wdiag", [128, CCH * K], f32, isOutput=False)
    wpwt_d = nc.declare_dram_parameter(
        "wpwt", [128, CCH, CCH, 128], f32, isOutput=False
    )
    bdw_d = nc.declare_dram_parameter("bdw", [128, CCH], f32, isOutput=False)
    bpw_d = nc.declare_dram_parameter("bpw", [128, CCH], f32, isOutput=False)
    out_d = nc.declare_dram_parameter("out", [C, OUT_ROWS], f32, isOutput=True)
    out_v = out_d.rearrange("(h q) r -> q h r", h=CCH)  # [128, 4, OUT_ROWS]
    xt_pe_v = xt_d.rearrange("(g j p) w -> g p j w", g=2, j=PE_CH)[0]
    xt_dve_v = xt_d.rearrange("(g j p) w -> g p j w", g=2, j=CCH - PE_CH)[1]

    with TileContext(nc) as tc:
        with (
            tc.tile_pool(name="consts", bufs=1) as cpool,
            tc.tile_pool(name="xtp", bufs=3) as xtp_pool,
            tc.tile_pool(name="xtv", bufs=2) as xtv_pool,
            tc.tile_pool(name="dwt", bufs=3) as dwt_pool,
            tc.tile_pool(name="dwtv", bufs=2) as dwtv_pool,
            tc.tile_pool(name="outsb", bufs=3) as out_pool,
            tc.tile_pool(name="dwps", bufs=3, space="PSUM") as dw_psum,
            tc.tile_pool(name="outps", bufs=3, space="PSUM") as out_psum,
        ):
            # small consts on sync ring first
            wdiag_src = cpool.tile([128, CCH * K], f32)
            nc.sync.dma_start(out=wdiag_src[:], in_=wdiag_d[:])
            bdw = cpool.tile([128, CCH], f32)
            nc.sync.dma_start(out=bdw[:], in_=bdw_d[:])
            bpw = cpool.tile([128, CCH], f32)
            nc.sync.dma_start(out=bpw[:], in_=bpw_d[:])
            wpwt_f = cpool.tile([128, CCH, CCH, 128], f32)
            nc.sync.dma_start(out=wpwt_f[:], in_=wpwt_d[:])

            # gpsimd builds f32r constants (keeps DVE free, PE unblocked early)
            ident = cpool.tile([128, 128], f32)
            make_identity(nc, ident[:])
            diag = cpool.tile([128, PE_CH * K * 128], f32r)
            for u in range(PE_CH * K):
                nc.gpsimd.tensor_scalar_mul(
                    diag[:, u * 128 : (u + 1) * 128],
                    ident[:],
                    wdiag_src[:, u : u + 1],
                )
            wpwt = cpool.tile([128, CCH, CCH, 128], f32r)
            nc.gpsimd.tensor_copy(wpwt[:], wpwt_f[:])

            NDV = CCH - PE_CH

            def load_pe(lb):
                blk = BLKS[lb]
                t = xtp_pool.tile([128, PE_CH, blk + 8], f32r, tag="xtp", name=f"xtp{lb}")
                nc.gpsimd.dma_start(
                    out=t[:, :, 0 : blk + 4],
                    in_=xt_pe_v[:, :, 512 * lb : 512 * lb + blk + 4],
                )
                return t

            def load_dve(sb):
                sblk = SBS[sb]
                t = xtv_pool.tile([128, NDV, sblk + 8], f32, tag="xtv", name=f"xtv{sb}")
                nc.sync.dma_start(
                    out=t[:, :, 0 : sblk + 4],
                    in_=xt_dve_v[:, :, 1024 * sb : 1024 * sb + sblk + 4],
                )
                return t

            def conv_dve(sb, xtv):
                sblk = SBS[sb]
                dt_ = dwtv_pool.tile([128, NDV, sblk], f32r, tag="dwtv", name=f"dwtv{sb}")
                for jj in range(NDV):
                    j = PE_CH + jj
                    nc.vector.tensor_scalar(
                        dt_[:, jj, :],
                        xtv[:, jj, 0:sblk],
                        wdiag_src[:, j * K : j * K + 1],
                        bdw[:, j : j + 1],
                        op0=Alu.mult,
                        op1=Alu.add,
                    )
                    for k in range(1, K):
                        nc.vector.scalar_tensor_tensor(
                            dt_[:, jj, :],
                            xtv[:, jj, k : k + sblk],
                            wdiag_src[:, j * K + k : j * K + k + 1],
                            dt_[:, jj, :],
                            op0=Alu.mult,
                            op1=Alu.add,
                        )
                return dt_

            def conv_pe(lb, xtp):
                blk = BLKS[lb]
                dwt = []
                for j in range(PE_CH):
                    ps = dw_psum.tile([128, blk], f32, tag="dwps", name=f"ps{j}_{lb}")
                    for k in range(K):
                        nc.tensor.matmul(
                            ps[:],
                            lhsT=diag[:, (j * K + k) * 128 : (j * K + k + 1) * 128],
                            rhs=xtp[:, j, k : k + blk],
                            start=(k == 0),
                            stop=(k == K - 1),
                        )
                    dt_ = dwt_pool.tile([128, blk], f32r, tag=f"dwt{j}", name=f"dwt{j}_{lb}")
                    nc.scalar.add(dt_[:], ps[:], bdw[:, j : j + 1])
                    dwt.append(dt_)
                return dwt

            def pointwise(lb, dwt_pe, dwtv, off):
                blk = BLKS[lb]
                ob = out_pool.tile([128, CCH, blk], f32, tag="outsb", name=f"ob{lb}")
                for dch in range(CCH):
                    po = out_psum.tile([128, blk], f32, tag="outps", name=f"po{dch}_{lb}")
                    for j in range(CCH):
                        rhs = (
                            dwt_pe[j][:, 0:blk]
                            if j < PE_CH
                            else dwtv[:, j - PE_CH, off : off + blk]
                        )
                        nc.tensor.matmul(
                            po[:],
                            lhsT=wpwt[:, j, dch, :],
                            rhs=rhs,
                            start=(j == 0),
                            stop=(j == CCH - 1),
                        )
                    nc.scalar.add(ob[:, dch, :], po[:], bpw[:, dch : dch + 1])
                nc.sync.dma_start(
                    out=out_v[:, :, 512 * lb : 512 * lb + blk], in_=ob[:]
                )

            for sb in range(len(SBS)):
                xtv = load_dve(sb)
                lbs = [2 * sb] + ([2 * sb + 1] if SBS[sb] == 1024 else [])
                xtps = [load_pe(lb) for lb in lbs]
                dtv = conv_dve(sb, xtv)
                for lb, xtp in zip(lbs, xtps):
                    dwt_pe = conv_pe(lb, xtp)
                    pointwise(lb, dwt_pe, dtv, 512 * (lb - 2 * sb))

    nc.finalize()
    return nc


def _get_nc():
    if "nc" not in _cached:
        _cached["nc"] = _build_nc()
    return _cached["nc"]


def _analyze(segment_boundaries):
    starts = segment_boundaries[..., 0].astype(np.int64)  # [B,S]
    ends = segment_boundaries[..., 1].astype(np.int64)
    pos = np.arange(L)
    in_seg = (pos[None, None, :] >= starts[..., None]) & (
        pos[None, None, :] < ends[..., None]
    )  # [B,S,L]
    covered = in_seg.any(axis=1)
    seg_id = np.where(covered, in_seg.argmax(axis=1), -1)  # [B,L]
    return covered, seg_id


def kernel(x, segment_boundaries, w_dw, b_dw, w_pw, b_pw):
    from concourse.bass_utils import run_bass_kernel_spmd

    x = np.asarray(x, dtype=np.float32)
    sb = np.asarray(segment_boundaries)
    w_dw = np.asarray(w_dw, dtype=np.float32)
    b_dw = np.asarray(b_dw, dtype=np.float32)
    w_pw = np.asarray(w_pw, dtype=np.float32)
    b_pw = np.asarray(b_pw, dtype=np.float32)

    covered, seg_id = _analyze(sb)

    # ---- run decomposition + stream build ----
    pieces = []          # [len, C] chunks
    src_b_parts = []
    src_l_parts = []
    run_start_of = np.full((B, L), -1, np.int64)  # run start index per covered pos
    for b in range(B):
        sid = seg_id[b]
        change = np.nonzero(np.diff(sid) != 0)[0] + 1
        bounds = np.concatenate([[0], change, [L]])
        for s, e in zip(bounds[:-1], bounds[1:]):
            if sid[s] < 0:
                continue
            run_start_of[b, s:e] = s
            pieces.append(np.zeros((4, C), np.float32))
            src_b_parts.append(np.full(4, -1, np.int64))
            src_l_parts.append(np.full(4, -1, np.int64))
            pieces.append(x[b, s:e])
            src_b_parts.append(np.full(e - s, b, np.int64))
            src_l_parts.append(np.arange(s, e, dtype=np.int64))
    if pieces:
        stream = np.concatenate(pieces, axis=0)
        src_b = np.concatenate(src_b_parts)
        src_l = np.concatenate(src_l_parts)
    else:
        stream = np.zeros((0, C), np.float32)
        src_b = np.zeros(0, np.int64)
        src_l = np.zeros(0, np.int64)
    T = stream.shape[0]
    Q = -(-T // NCORES) if T else 1
    assert Q + 4 <= XT_W - 8, f"stream quota {Q} too large"

    # ---- per-core inputs ----
    wdiag = np.ascontiguousarray(
        w_dw.reshape(CCH, 128, K).transpose(1, 0, 2).reshape(128, CCH * K)
    )
    # wpwt[p, j, dch, q] = w_pw[dch*128+q, j*128+p]
    wpwt = np.ascontiguousarray(
        w_pw.reshape(CCH, 128, CCH, 128).transpose(3, 2, 0, 1)
    )
    bdwr = np.ascontiguousarray(b_dw.reshape(CCH, 128).T)  # [128, CCH]
    bpwr = np.ascontiguousarray(b_pw.reshape(CCH, 128).T)  # [128, CCH]

    in_maps = []
    spans = []
    for i in range(NCORES):
        lo, hi = i * Q, min((i + 1) * Q, T)
        lo = min(lo, T)
        spans.append((lo, hi))
        buf = np.zeros((XT_W, C), np.float32)
        if hi > lo:
            hlo = max(0, lo - 4)
            buf[4 - (lo - hlo) : 4 + (hi - lo)] = stream[hlo:hi]
        in_maps.append(
            {
                "xt": np.ascontiguousarray(buf.T),
                "wdiag": wdiag,
                "wpwt": wpwt,
                "bdw": bdwr,
                "bpw": bpwr,
            }
        )

    nc = _get_nc()
    res = run_bass_kernel_spmd(nc, in_maps, list(range(NCORES)))

    # ---- gather (device out is [C, OUT_ROWS], transposed) ----
    so_out = np.zeros((T, C), np.float32)
    for i, (lo, hi) in enumerate(spans):
        if hi > lo:
            so_out[lo:hi] = res.results[i]["out"][:, : hi - lo].T
    out = np.zeros((B, L, C), np.float32)
    mask = src_l >= 0
    out[src_b[mask], src_l[mask]] = so_out[mask]

    # ---- general-case sparse correction (pairwise mask vs run mask) ----
    # reference: m_ref_d[l] = covered[l] & l>=d & seg_id[l-d]==seg_id[l]
    # device computed run mask: m_run_d[l] = covered[l] & (l - run_start >= d)
    need = []
    for d in range(1, K):
        m_ref = np.zeros((B, L), bool)
        m_ref[:, d:] = covered[:, d:] & (seg_id[:, d:] == seg_id[:, :-d])
        m_run = covered & (np.arange(L)[None, :] - run_start_of >= d)
        diff = m_ref.astype(np.int8) - m_run.astype(np.int8)
        if np.any(diff):
            bs, ls = np.nonzero(diff)
            need.append((d, bs, ls, diff[bs, ls].astype(np.float32)))
    if need:
        for d, bs, ls, sgn in need:
            xv = x[bs, ls - d, :]  # ls >= d guaranteed where masks differ
            delta_dw = xv * w_dw[None, :, K - 1 - d] * sgn[:, None]
            out[bs, ls, :] += delta_dw @ w_pw.T

    return out


# revision 19
# speedup vs baseline: 1.2611x; 1.2611x over previous
"""Causal segment-masked depthwise conv (K=5) + pointwise conv, 8-core SPMD.

Strategy:
  Host: pack each batch row's segments into a global stream with 4 zeros
  before each segment (plain causal conv on the stream == per-segment
  left-zero-padded conv), split the stream evenly across 8 cores with a
  4-element halo, and pre-transpose each core's slab to [C, stream].
  Device: depthwise conv for channel chunks 0-1 as diag-stationary fp32r
  matmuls in PSUM (+ ACT bias copy), chunks 2-3 on DVE via fused
  scalar_tensor_tensor over 1024-wide superblocks; pointwise matmul with
  w_pw^T chunks stationary and dwT moving -> transposed [d, l] PSUM
  output, ACT adds b_pw as per-partition bias, batched store. Host
  transposes back during gather and applies a sparse general-case
  correction for exotic segment overlap patterns (empty for contiguous
  partitions).
"""

import os
import sys

sys.path.insert(0, "/opt/trn_rl_repo")

import numpy as np

B, L, C, K, S = 8, 4096, 512, 5, 8
NCORES = 8
CCH = C // 128          # 4 channel chunks
XT_W = 4360             # per-core stream buffer: 4 halo + 4348 capacity + pad
OUT_ROWS = 4352         # conv outputs for stream cols [4, 4356)
NBLK = 9                # 8 blocks of 512 + 1 of 256
BLKS = [512] * 8 + [256]
SBS = [1024] * 4 + [256]   # DVE superblocks
PE_CH = 2               # channel chunks 0..PE_CH-1 on PE, rest on DVE

_cached = {}


def _build_nc():
    import concourse.mybir as mybir
    from concourse import bacc
    from concourse.tile import TileContext

    f32 = mybir.dt.float32
    f32r = mybir.dt.float32r
    Alu = mybir.AluOpType

    nc = bacc.Bacc(num_swdge_queues=2)
    xt_d = nc.declare_dram_parameter("xt", [C, XT_W], f32, isOutput=False)
    wdiag_d = nc.declare_dram_parameter("wdiag", [128, CCH * K], f32, isOutput=False)
    diag_d = nc.declare_dram_parameter(
        "diag", [128, PE_CH * K * 128], f32, isOutput=False
    )
    wpwt_d = nc.declare_dram_parameter(
        "wpwt", [128, CCH, CCH, 128], f32, isOutput=False
    )
    bdw_d = nc.declare_dram_parameter("bdw", [128, CCH], f32, isOutput=False)
    bpw_d = nc.declare_dram_parameter("bpw", [128, CCH], f32, isOutput=False)
    out_d = nc.declare_dram_parameter("out", [C, OUT_ROWS], f32, isOutput=True)
    out_v = out_d.rearrange("(h q) r -> q h r", h=CCH)  # [128, 4, OUT_ROWS]
    xt_pe_v = xt_d.rearrange("(g j p) w -> g p j w", g=2, j=PE_CH)[0]
    xt_dve_v = xt_d.rearrange("(g j p) w -> g p j w", g=2, j=CCH - PE_CH)[1]

    with TileContext(nc) as tc:
        with (
            tc.tile_pool(name="consts", bufs=1) as cpool,
            tc.tile_pool(name="xtp", bufs=3) as xtp_pool,
            tc.tile_pool(name="xtv", bufs=2) as xtv_pool,
            tc.tile_pool(name="dwt", bufs=3) as dwt_pool,
            tc.tile_pool(name="dwtv", bufs=2) as dwtv_pool,
            tc.tile_pool(name="outsb", bufs=3) as out_pool,
            tc.tile_pool(name="dwps", bufs=3, space="PSUM") as dw_psum,
            tc.tile_pool(name="outps", bufs=3, space="PSUM") as out_psum,
        ):
            # small consts on sync ring first
            wdiag_src = cpool.tile([128, CCH * K], f32)
            nc.sync.dma_start(out=wdiag_src[:], in_=wdiag_d[:])
            bdw = cpool.tile([128, CCH], f32)
            nc.sync.dma_start(out=bdw[:], in_=bdw_d[:])
            bpw = cpool.tile([128, CCH], f32)
            nc.sync.dma_start(out=bpw[:], in_=bpw_d[:])
            diag_f = cpool.tile([128, PE_CH * K * 128], f32)
            nc.sync.dma_start(out=diag_f[:], in_=diag_d[:])
            wpwt_f = cpool.tile([128, CCH, CCH, 128], f32)
            nc.sync.dma_start(out=wpwt_f[:], in_=wpwt_d[:])

            # ACT copy-casts host-prebuilt f32 constants to f32r
            diag = cpool.tile([128, PE_CH * K * 128], f32r)
            nc.scalar.copy(diag[:], diag_f[:])
            wpwt = cpool.tile([128, CCH, CCH, 128], f32r)
            nc.scalar.copy(wpwt[:], wpwt_f[:])

            NDV = CCH - PE_CH

            def load_pe(lb):
                blk = BLKS[lb]
                t = xtp_pool.tile([128, PE_CH, blk + 8], f32r, tag="xtp", name=f"xtp{lb}")
                nc.gpsimd.dma_start(
                    out=t[:, :, 0 : blk + 4],
                    in_=xt_pe_v[:, :, 512 * lb : 512 * lb + blk + 4],
                )
                return t

            def load_dve(sb):
                sblk = SBS[sb]
                ts = []
                for jj in range(NDV):
                    t = xtv_pool.tile(
                        [128, sblk + 8], f32, tag=f"xtv{jj}", name=f"xtv{jj}_{sb}"
                    )
                    nc.sync.dma_start(
                        out=t[:, 0 : sblk + 4],
                        in_=xt_dve_v[:, jj, 1024 * sb : 1024 * sb + sblk + 4],
                    )
                    ts.append(t)
                return ts

            def conv_dve(sb, xtv):
                sblk = SBS[sb]
                dts = []
                for jj in range(NDV):
                    j = PE_CH + jj
                    dt_ = dwtv_pool.tile(
                        [128, sblk], f32r, tag=f"dwtv{jj}", name=f"dwtv{jj}_{sb}"
                    )
                    nc.vector.tensor_scalar(
                        dt_[:],
                        xtv[jj][:, 0:sblk],
                        wdiag_src[:, j * K : j * K + 1],
                        bdw[:, j : j + 1],
                        op0=Alu.mult,
                        op1=Alu.add,
                    )
                    for k in range(1, K):
                        nc.vector.scalar_tensor_tensor(
                            dt_[:],
                            xtv[jj][:, k : k + sblk],
                            wdiag_src[:, j * K + k : j * K + k + 1],
                            dt_[:],
                            op0=Alu.mult,
                            op1=Alu.add,
                        )
                    dts.append(dt_)
                return dts

            def conv_pe(lb, xtp):
                blk = BLKS[lb]
                dwt = []
                for j in range(PE_CH):
                    ps = dw_psum.tile([128, blk], f32, tag="dwps", name=f"ps{j}_{lb}")
                    for k in range(K):
                        nc.tensor.matmul(
                            ps[:],
                            lhsT=diag[:, (j * K + k) * 128 : (j * K + k + 1) * 128],
                            rhs=xtp[:, j, k : k + blk],
                            start=(k == 0),
                            stop=(k == K - 1),
                        )
                    dt_ = dwt_pool.tile([128, blk], f32r, tag=f"dwt{j}", name=f"dwt{j}_{lb}")
                    nc.scalar.add(dt_[:], ps[:], bdw[:, j : j + 1])
                    dwt.append(dt_)
                return dwt

            def pointwise(lb, dwt_pe, dwtv, off):
                blk = BLKS[lb]
                ob = out_pool.tile([128, CCH, blk], f32, tag="outsb", name=f"ob{lb}")
                for dch in range(CCH):
                    po = out_psum.tile([128, blk], f32, tag="outps", name=f"po{dch}_{lb}")
                    for j in range(CCH):
                        rhs = (
                            dwt_pe[j][:, 0:blk]
                            if j < PE_CH
                            else dwtv[j - PE_CH][:, off : off + blk]
                        )
                        nc.tensor.matmul(
                            po[:],
                            lhsT=wpwt[:, j, dch, :],
                            rhs=rhs,
                            start=(j == 0),
                            stop=(j == CCH - 1),
                        )
                    nc.scalar.add(ob[:, dch, :], po[:], bpw[:, dch : dch + 1])
                nc.sync.dma_start(
                    out=out_v[:, :, 512 * lb : 512 * lb + blk], in_=ob[:]
                )

            for sb in range(len(SBS)):
                xtv = load_dve(sb)
                lbs = [2 * sb] + ([2 * sb + 1] if SBS[sb] == 1024 else [])
                xtps = [load_pe(lb) for lb in lbs]
                dtv = conv_dve(sb, xtv)
                for lb, xtp in zip(lbs, xtps):
                    dwt_pe = conv_pe(lb, xtp)
                    pointwise(lb, dwt_pe, dtv, 512 * (lb - 2 * sb))

    nc.finalize()
    return nc


def _get_nc():
    if "nc" not in _cached:
        _cached["nc"] = _build_nc()
    return _cached["nc"]


def _analyze(segment_boundaries):
    starts = segment_boundaries[..., 0].astype(np.int64)  # [B,S]
    ends = segment_boundaries[..., 1].astype(np.int64)
    pos = np.arange(L)
    in_seg = (pos[None, None, :] >= starts[..., None]) & (
        pos[None, None, :] < ends[..., None]
    )  # [B,S,L]
    covered = in_seg.any(axis=1)
    seg_id = np.where(covered, in_seg.argmax(axis=1), -1)  # [B,L]
    return covered, seg_id


def kernel(x, segment_boundaries, w_dw, b_dw, w_pw, b_pw):
    from concourse.bass_utils import run_bass_kernel_spmd

    x = np.asarray(x, dtype=np.float32)
    sb = np.asarray(segment_boundaries)
    w_dw = np.asarray(w_dw, dtype=np.float32)
    b_dw = np.asarray(b_dw, dtype=np.float32)
    w_pw = np.asarray(w_pw, dtype=np.float32)
    b_pw = np.asarray(b_pw, dtype=np.float32)

    covered, seg_id = _analyze(sb)

    # ---- run decomposition + stream build ----
    pieces = []          # [len, C] chunks
    src_b_parts = []
    src_l_parts = []
    run_start_of = np.full((B, L), -1, np.int64)  # run start index per covered pos
    for b in range(B):
        sid = seg_id[b]
        change = np.nonzero(np.diff(sid) != 0)[0] + 1
        bounds = np.concatenate([[0], change, [L]])
        for s, e in zip(bounds[:-1], bounds[1:]):
            if sid[s] < 0:
                continue
            run_start_of[b, s:e] = s
            pieces.append(np.zeros((4, C), np.float32))
            src_b_parts.append(np.full(4, -1, np.int64))
            src_l_parts.append(np.full(4, -1, np.int64))
            pieces.append(x[b, s:e])
            src_b_parts.append(np.full(e - s, b, np.int64))
            src_l_parts.append(np.arange(s, e, dtype=np.int64))
    if pieces:
        stream = np.concatenate(pieces, axis=0)
        src_b = np.concatenate(src_b_parts)
        src_l = np.concatenate(src_l_parts)
    else:
        stream = np.zeros((0, C), np.float32)
        src_b = np.zeros(0, np.int64)
        src_l = np.zeros(0, np.int64)
    T = stream.shape[0]
    Q = -(-T // NCORES) if T else 1
    assert Q + 4 <= XT_W - 8, f"stream quota {Q} too large"

    # ---- per-core inputs ----
    wdiag = np.ascontiguousarray(
        w_dw.reshape(CCH, 128, K).transpose(1, 0, 2).reshape(128, CCH * K)
    )
    # prebuilt diagonal stationaries for the PE conv chunks
    diag = np.zeros((128, PE_CH * K * 128), np.float32)
    for u in range(PE_CH * K):
        np.fill_diagonal(diag[:, u * 128 : (u + 1) * 128], wdiag[:, u])
    # wpwt[p, j, dch, q] = w_pw[dch*128+q, j*128+p]
    wpwt = np.ascontiguousarray(
        w_pw.reshape(CCH, 128, CCH, 128).transpose(3, 2, 0, 1)
    )
    bdwr = np.ascontiguousarray(b_dw.reshape(CCH, 128).T)  # [128, CCH]
    bpwr = np.ascontiguousarray(b_pw.reshape(CCH, 128).T)  # [128, CCH]

    in_maps = []
    spans = []
    for i in range(NCORES):
        lo, hi = i * Q, min((i + 1) * Q, T)
        lo = min(lo, T)
        spans.append((lo, hi))
        buf = np.zeros((XT_W, C), np.float32)
        if hi > lo:
            hlo = max(0, lo - 4)
            buf[4 - (lo - hlo) : 4 + (hi - lo)] = stream[hlo:hi]
        in_maps.append(
            {
                "xt": np.ascontiguousarray(buf.T),
                "wdiag": wdiag,
                "diag": diag,
                "wpwt": wpwt,
                "bdw": bdwr,
                "bpw": bpwr,
            }
        )

    nc = _get_nc()
    res = run_bass_kernel_spmd(nc, in_maps, list(range(NCORES)))

    # ---- gather (device out is [C, OUT_ROWS], transposed) ----
    so_out = np.zeros((T, C), np.float32)
    for i, (lo, hi) in enumerate(spans):
        if hi > lo:
            so_out[lo:hi] = res.results[i]["out"][:, : hi - lo].T
    out = np.zeros((B, L, C), np.float32)
    mask = src_l >= 0
    out[src_b[mask], src_l[mask]] = so_out[mask]

    # ---- general-case sparse correction (pairwise mask vs run mask) ----
    # reference: m_ref_d[l] = covered[l] & l>=d & seg_id[l-d]==seg_id[l]
    # device computed run mask: m_run_d[l] = covered[l] & (l - run_start >= d)
    need = []
    for d in range(1, K):
        m_ref = np.zeros((B, L), bool)
        m_ref[:, d:] = covered[:, d:] & (seg_id[:, d:] == seg_id[:, :-d])
        m_run = covered & (np.arange(L)[None, :] - run_start_of >= d)
        diff = m_ref.astype(np.int8) - m_run.astype(np.int8)
        if np.any(diff):
            bs, ls = np.nonzero(diff)
            need.append((d, bs, ls, diff[bs, ls].astype(np.float32)))
    if need:
        for d, bs, ls, sgn in need:
            xv = x[bs, ls - d, :]  # ls >= d guaranteed where masks differ
            delta_dw = xv * w_dw[None, :, K - 1 - d] * sgn[:, None]
            out[bs, ls, :] += delta_dw @ w_pw.T

    return out
